# revision 1
# baseline (speedup 1.0000x reference)
"""Trainium2 Bass kernel for SKalmanNet GSS (dense GEMV chain, batch=1).

Strategy (8 NeuronCores):
  - The two branches (Pk from l1/gru1/l2, Sk from l3/gru2/l4) are independent
    and have identical shapes -> one SPMD program, cores 0-3 run branch P,
    cores 4-7 run branch S (replica_groups [[0,1,2,3],[4,5,6,7]]).
  - Within a group of 4 cores: tensor-parallel row-sharding of every weight
    matrix; AllGather of the small activation vector after l1, after the GRU
    cell, and after l2_W1.  The final l2_W2 output shard goes straight to the
    per-core output (host concatenates).
  - Each matvec is computed with the activation chunk as the *stationary*
    matmul operand ([128,1]) and the (host-pre-transposed) weight tile as the
    *moving* operand ([128,<=512]) so the TensorEngine streams weights at
    ~512 MACs/row-cycle with negligible LDWEIGHTS cost.  PSUM accumulates
    over input chunks.
  - Weights are re-laid-out on the host so every DMA is a large fully
    contiguous HBM read ([128, G*Ms] tiles, ~0.5-2 MB each).
"""

import os

import numpy as np

X = 32
Y = 32
H1 = 5120          # l1 rows
HID = 2048         # gru hidden
H2 = 4096          # l2_W1 rows
OUT = 1024         # l2_W2 rows (X*X)
IN = 1120          # input vec (2X + Y + XY)
INP = 1152         # padded to 9*128 (slot 1120 = 1.0 for folded l1 bias)

NCORES = 8
TP = 4             # cores per branch

# per-core shard sizes
M_L1 = H1 // TP        # 1280
M_G = 3 * (HID // TP)  # 1536  (r|z|n gate rows, 512 each)
M_W1 = H2 // TP        # 1024
M_W2 = OUT // TP       # 256
HSH = HID // TP        # 512

# (K chunks of 128 input dims, chunks per DMA group, output cols per chunk)
L_L1 = (INP // 128, 1, M_L1)    # (9, 1, 1280)  -> dram [9, 128, 1280]
L_IH = (H1 // 128, 4, M_G)      # (40, 4, 1536) -> [10, 128, 6144]
L_HH = (HID // 128, 4, M_G)     # (16, 4, 1536) -> [4, 128, 6144]
L_W1 = (HID // 128, 4, M_W1)    # (16, 4, 1024) -> [4, 128, 4096]
L_W2 = (H2 // 128, 16, M_W2)    # (32, 16, 256) -> [2, 128, 4096]

# consts layout (f32, [1, 3840]):
#   b_rz(1024) | bih_n(512) | bhh_n(512) | b1(1024) | b2(256) | h_shard(512)
C_BRZ = 0
C_BIHN = 1024
C_BHHN = 1536
C_B1 = 2048
C_B2 = 3072
C_HSH = 3328
C_TOT = 3840

WDT_NAME = os.environ.get("KERNEL_WDT", "float16")  # weight/activation dtype
# weight-stream ring depth: slot is [128, 6144] of wdt per buf
_DEF_WBUFS = {"float32": 6, "float16": 12, "bfloat16": 12}[WDT_NAME]
WBUFS = int(os.environ.get("KERNEL_WBUFS", str(_DEF_WBUFS)))

_CACHE = {}


def _np_wdt():
    return {"float32": np.float32, "float16": np.float16,
            "bfloat16": None}[WDT_NAME]


def _build_nc():
    import concourse.bass as bass  # noqa: F401
    import concourse.mybir as mybir
    import concourse.tile as tile
    from concourse import bacc

    f32 = mybir.dt.float32
    wdt = {"float32": mybir.dt.float32,
           "float16": mybir.dt.float16,
           "bfloat16": mybir.dt.bfloat16}[WDT_NAME]

    nc = bacc.Bacc("TRN2", target_bir_lowering=False, debug=False,
                   num_devices=NCORES)

    # x / hn arrive pre-chunked ([p, k] with element k*128+p at [p, k]) so
    # the SBUF load is a plain 2D copy instead of a 2-byte-per-partition
    # scatter.
    x_d = nc.dram_tensor("x", [128, INP // 128], wdt, kind="ExternalInput")
    hn_d = nc.dram_tensor("hn", [128, HID // 128], wdt, kind="ExternalInput")
    consts_d = nc.dram_tensor("consts", [1, C_TOT], f32, kind="ExternalInput")

    def wtensor(name, spec):
        K, G, Ms = spec
        return nc.dram_tensor(name, [K // G, 128, G * Ms], wdt,
                              kind="ExternalInput")

    wl1_d = wtensor("wl1", L_L1)
    wih_d = wtensor("wih", L_IH)
    whh_d = wtensor("whh", L_HH)
    w1_d = wtensor("w1", L_W1)
    w2_d = wtensor("w2", L_W2)
    out_d = nc.dram_tensor("out", [1, M_W2], f32, kind="ExternalOutput")

    AF = mybir.ActivationFunctionType
    groups = [[0, 1, 2, 3], [4, 5, 6, 7]]
    agc = [0]  # unique-name counter for collective bounce tiles

    with tile.TileContext(nc) as tc, \
         tc.tile_pool(name="w", bufs=WBUFS) as wpool, \
         tc.tile_pool(name="act", bufs=1) as apool, \
         tc.tile_pool(name="ps", bufs=8, space="PSUM") as ppool, \
         tc.tile_pool(name="dram", bufs=1, space="DRAM") as dpool:

        consts = apool.tile([1, C_TOT], f32, tag="consts", name="consts_sb")
        nc.gpsimd.dma_start(consts, consts_d.ap())
        x0 = apool.tile([128, INP // 128], wdt, tag="x0", name="x0")
        nc.gpsimd.dma_start(x0, x_d.ap())
        hx = apool.tile([128, HID // 128], wdt, tag="hx", name="hx")
        nc.gpsimd.dma_start(hx, hn_d.ap())

        # Preload ACT LUTs so sigmoid/tanh don't pay table-load latency on
        # the critical path.
        warm = apool.tile([1, 32], f32, tag="warm", name="warm")
        nc.vector.memset(warm, 0.0)
        nc.scalar.activation(warm, warm, AF.Sigmoid)
        nc.scalar.activation(warm, warm, AF.Tanh)
        nc.scalar.activation(warm, warm, AF.Relu)

        # cross-chain reduction operand: every copied psum row holds its
        # chain's full sum (broadcast-32 stationary), so summing the first
        # 32*nch racc rows with weight 1/32 yields the chain total exactly.
        sel = apool.tile([128, 1], wdt, tag="sel", name="sel")
        nc.vector.memset(sel, 1.0 / 32.0)

        def gemv(x_sb, wt_d, spec, xmap=None, nch=2):
            """psum tiles [1,<=512] = W_shard @ x.

            The K input chunks are split round-robin over `nch` PE
            column-group chains (each chain accumulates in its own PSUM
            bank at a distinct quadrant via tile_position) so the chains
            stream through the array concurrently.  The stationary
            x-column is broadcast to 32 array columns (all 32 psum rows of
            a quadrant hold the chain sum), and a final 1-column matmul
            with sel_n adds one row per chain into the [1,mw] result.
            nch is bounded by PSUM banks: nm*nch + transients <= 8.
            """
            K, G, Ms = spec
            nm = (Ms + 511) // 512
            mts = [(i * 512, min(512, Ms - i * 512)) for i in range(nm)]
            ci = agc[0]
            agc[0] += 1
            # accs[mi][c] — one psum bank per (m-tile, chain); chain c
            # occupies quadrant rows 32c..32c+32
            accs = [[ppool.tile([128, mw], f32, tag="ps",
                                name=f"acc{ci}_{i}_{c}")
                     for c in range(nch)]
                    for i, (_, mw) in enumerate(mts)]
            last_k = [max(k for k in range(K) if k % nch == c)
                      for c in range(nch)]
            for g in range(K // G):
                wt = wpool.tile([128, G * Ms], wdt, tag="w", name="wt")
                # weights stream on the SP HWDGE ring only; the ACT ring is
                # kept clear for latency-critical bounce/transpose DMAs
                nc.sync.dma_start(wt, wt_d.ap()[g])
                for j in range(G):
                    k = g * G + j
                    kk = xmap(k) if xmap else k
                    c = k % nch
                    r0 = 32 * c
                    xbc = x_sb[:, kk:kk + 1].broadcast_to([128, 32])
                    for mi, (m0, mw) in enumerate(mts):
                        nc.tensor.matmul(
                            accs[mi][c][r0:r0 + 32, :],
                            xbc,
                            wt[:, j * Ms + m0: j * Ms + m0 + mw],
                            start=(k == c), stop=(k == last_k[c]),
                            tile_position=(0, r0),
                        )
            ps = []
            for mi, (m0, mw) in enumerate(mts):
                racc = apool.tile([128, 512], wdt, tag="racc", bufs=3,
                                  name=f"racc{ci}_{mi}")
                for c in range(nch):
                    r0 = 32 * c
                    nc.scalar.copy(racc[r0:r0 + 32, :mw],
                                   accs[mi][c][r0:r0 + 32, :])
                op = ppool.tile([1, mw], f32, tag="ps", name=f"pso{ci}_{mi}")
                nc.tensor.matmul(op[:, :], sel[:32 * nch, :],
                                 racc[:32 * nch, :mw],
                                 start=True, stop=True)
                ps.append(op)
            return ps, mts

        def allgather(y_sb, n, n_pad, nm_name):
            """y_sb [1,n_pad] shard (cols n..n_pad zero) -> SBUF [128, 4*n_pad/128]
            chunk-layout gathered vector (per-rank tail chunks zero)."""
            i = agc[0]
            agc[0] += 1
            agin = dpool.tile([1, n_pad], y_sb.dtype, tag=f"agi{i}",
                              name=f"agin_{nm_name}")
            agout = dpool.tile([TP, n_pad], y_sb.dtype, tag=f"ago{i}",
                               name=f"agout_{nm_name}")
            nc.scalar.dma_start(agin, y_sb)
            nc.gpsimd.collective_compute(
                "AllGather", mybir.AluOpType.bypass,
                replica_groups=groups,
                ins=[agin.opt()], outs=[agout.opt()],
            )
            kt = TP * n_pad // 128
            xt = apool.tile([128, kt], y_sb.dtype,
                            tag=f"x{i}", name=f"x_{nm_name}")
            if mybir.dt.size(wdt) == 2:
                # hardware X-bar transpose: DRAM [kt,128] -> SBUF [128,kt]
                nc.scalar.dma_start_transpose(
                    xt, agout.rearrange("r (k p) -> (r k) p", p=128))
            else:
                nc.gpsimd.dma_start(
                    xt, agout.rearrange("r (c p) -> p (r c)", p=128))
            return xt

        # ---- l1 first: it feeds the first AllGather (critical path)
        #      (bias folded into row 1120 of wl1)
        ps1, mts1 = gemv(x0, wl1_d, L_L1)
        # ---- GRU hidden-side matvec: depends only on hn; its matmuls fill
        #      the PE while the l1 AllGather is in flight
        gh, _ = gemv(hx, whh_d, L_HH)            # 3x [1,512] (r,z,n)
        # stash gh in SBUF: frees its psum banks before gi's chains open,
        # and lets the gate math read it as the second DVE operand
        ghs = apool.tile([1, 1536], f32, tag="ghs", name="ghs")
        for gg in range(3):
            nc.scalar.copy(ghs[:, gg * 512:(gg + 1) * 512], gh[gg][:, :])

        # padded AG staging (pad region zeroed once, off critical path)
        PL1 = 1536
        xmap1 = lambda k: (k // 10) * (PL1 // 128) + k % 10   # noqa: E731
        xmap3 = lambda k: (k // 8) * (PL1 // 128) + k % 8     # noqa: E731
        y1 = apool.tile([1, PL1], wdt, tag="y1", name="y1")
        nc.vector.memset(y1[:, M_L1:PL1], 0.0)
        for mi, (m0, mw) in enumerate(mts1):
            nc.scalar.activation(y1[:, m0:m0 + mw], ps1[mi][:, :], AF.Relu)
        x1 = allgather(y1, M_L1, PL1, "l1")      # [128, 48]

        # ---- GRU input-side matvec
        gi, _ = gemv(x1, wih_d, L_IH, xmap=xmap1)  # 3x [1,512]

        # ---- GRU cell elementwise (shard of 512 hidden units)
        brz = consts[:, C_BRZ:C_BRZ + 1024]
        bihn = consts[:, C_BIHN:C_BIHN + 512]
        bhhn = consts[:, C_BHHN:C_BHHN + 512]
        hsh = consts[:, C_HSH:C_HSH + 512]

        t_r = apool.tile([1, 512], f32, tag="t_r", name="t_r")
        nc.vector.tensor_add(t_r, gi[0][:, :], brz[:, 0:512])
        nc.vector.tensor_add(t_r, t_r, ghs[:, 0:512])
        nc.scalar.activation(t_r, t_r, AF.Sigmoid)          # r

        t_z = apool.tile([1, 512], f32, tag="t_z", name="t_z")
        nc.vector.tensor_add(t_z, gi[1][:, :], brz[:, 512:1024])
        nc.vector.tensor_add(t_z, t_z, ghs[:, 512:1024])
        nc.scalar.activation(t_z, t_z, AF.Sigmoid)          # z

        t_hn = apool.tile([1, 512], f32, tag="t_hn", name="t_hn")
        nc.vector.tensor_add(t_hn, ghs[:, 1024:1536], bhhn)       # hn-gate pre
        t_n = apool.tile([1, 512], f32, tag="t_n", name="t_n")
        nc.vector.tensor_add(t_n, gi[2][:, :], bihn)        # in-gate pre
        nc.vector.tensor_mul(t_hn, t_r, t_hn)               # r * hn
        nc.vector.tensor_add(t_n, t_n, t_hn)
        nc.scalar.activation(t_n, t_n, AF.Tanh)             # n

        t_d = apool.tile([1, 512], f32, tag="t_d", name="t_d")
        nc.vector.tensor_sub(t_d, hsh, t_n)                 # h - n
        nc.vector.tensor_mul(t_d, t_z, t_d)                 # z*(h-n)
        hq = apool.tile([1, HSH], wdt, tag="hq", name="hq")
        nc.vector.tensor_add(hq, t_n, t_d)                  # h' = n + z*(h-n)

        x2 = allgather(hq, HSH, HSH, "gru")      # [128, 16]

        # ---- l2_W1: relu(W1 @ h' + b1)
        ps3, mts3 = gemv(x2, w1_d, L_W1, nch=3)
        y3 = apool.tile([1, PL1], wdt, tag="y3", name="y3")
        nc.vector.memset(y3[:, M_W1:PL1], 0.0)
        for mi, (m0, mw) in enumerate(mts3):
            nc.vector.tensor_add(y3[:, m0:m0 + mw], ps3[mi][:, :],
                                 consts[:, C_B1 + m0:C_B1 + m0 + mw])
            nc.scalar.activation(y3[:, m0:m0 + mw], y3[:, m0:m0 + mw],
                                 AF.Relu)
        x3 = allgather(y3, M_W1, PL1, "w1")      # [128, 48]

        # ---- l2_W2: W2 @ y3 + b2  -> per-core output shard
        ps4, _ = gemv(x3, w2_d, L_W2, xmap=xmap3, nch=4)  # 1x [1,256]
        yo = apool.tile([1, M_W2], f32, tag="yo", name="yo")
        nc.vector.tensor_add(yo, ps4[0][:, :], consts[:, C_B2:C_B2 + M_W2])
        nc.gpsimd.dma_start(out_d.ap(), yo)

    nc.finalize()
    return nc


def _pack(wt, K, G, Ms):
    """[K*128, Ms] input-dim-major transposed weight -> [K//G, 128, G*Ms]."""
    return np.ascontiguousarray(
        wt.reshape(K // G, G, 128, Ms).transpose(0, 2, 1, 3)
        .reshape(K // G, 128, G * Ms))


def _prep_core(r, xvec, hn, l1W, l1b, Wih, Whh, bih, bhh, W1, b1, W2, b2):
    npw = _np_wdt()
    f32 = np.float32

    rs = slice(r * M_L1, (r + 1) * M_L1)
    wt = np.zeros((INP, M_L1), f32)
    wt[:IN] = l1W[rs].T
    wt[IN] = l1b[rs]
    wl1 = _pack(wt, *L_L1)

    gsl = [slice(g * HID + r * HSH, g * HID + (r + 1) * HSH) for g in range(3)]
    gidx = np.concatenate([np.arange(s.start, s.stop) for s in gsl])
    wih = _pack(np.ascontiguousarray(Wih[gidx].T), *L_IH)
    whh = _pack(np.ascontiguousarray(Whh[gidx].T), *L_HH)
    w1 = _pack(np.ascontiguousarray(W1[r * M_W1:(r + 1) * M_W1].T), *L_W1)
    w2 = _pack(np.ascontiguousarray(W2[r * M_W2:(r + 1) * M_W2].T), *L_W2)

    bsum = bih + bhh
    consts = np.concatenate([
        bsum[gsl[0]], bsum[gsl[1]],          # b_rz
        bih[gsl[2]], bhh[gsl[2]],            # bih_n, bhh_n
        b1[r * M_W1:(r + 1) * M_W1],
        b2[r * M_W2:(r + 1) * M_W2],
        hn[r * HSH:(r + 1) * HSH],
    ]).astype(f32)[None]
    assert consts.shape[1] == C_TOT

    x = np.zeros(INP, f32)
    x[:IN] = xvec
    x[IN] = 1.0
    x_ch = np.ascontiguousarray(x.reshape(INP // 128, 128).T)
    hn_ch = np.ascontiguousarray(hn.reshape(HID // 128, 128).T)

    return {
        "x": x_ch.astype(npw), "hn": hn_ch.astype(npw), "consts": consts,
        "wl1": wl1.astype(npw), "wih": wih.astype(npw),
        "whh": whh.astype(npw), "w1": w1.astype(npw), "w2": w2.astype(npw),
    }


LAST_RESULT = None


def kernel(state_inno, observation_inno, diff_state, diff_obs,
           linearization_error, Jacobian,
           l1_W, l1_b, gru1_Wih, gru1_Whh, gru1_bih, gru1_bhh,
           l2_W1, l2_b1, l2_W2, l2_b2,
           l3_W, l3_b, gru2_Wih, gru2_Whh, gru2_bih, gru2_bhh,
           l4_W1, l4_b1, l4_W2, l4_b2, hn1, hn2):
    global LAST_RESULT
    from concourse.bass_utils import run_bass_kernel_spmd

    if "nc" not in _CACHE:
        _CACHE["nc"] = _build_nc()
    nc = _CACHE["nc"]

    a = lambda v: np.asarray(v, dtype=np.float32)
    input1 = np.concatenate([a(state_inno), a(diff_state),
                             a(linearization_error), a(Jacobian)]).reshape(-1)
    input2 = np.concatenate([a(observation_inno), a(diff_obs),
                             a(linearization_error), a(Jacobian)]).reshape(-1)

    branches = [
        (input1, a(hn1).reshape(-1), a(l1_W), a(l1_b).reshape(-1),
         a(gru1_Wih), a(gru1_Whh), a(gru1_bih).reshape(-1),
         a(gru1_bhh).reshape(-1), a(l2_W1), a(l2_b1).reshape(-1),
         a(l2_W2), a(l2_b2).reshape(-1)),
        (input2, a(hn2).reshape(-1), a(l3_W), a(l3_b).reshape(-1),
         a(gru2_Wih), a(gru2_Whh), a(gru2_bih).reshape(-1),
         a(gru2_bhh).reshape(-1), a(l4_W1), a(l4_b1).reshape(-1),
         a(l4_W2), a(l4_b2).reshape(-1)),
    ]
    in_maps = [_prep_core(c % TP, *branches[c // TP]) for c in range(NCORES)]

    kwargs = {}
    if os.environ.get("KERNEL_TRACE"):
        cores = os.environ.get("KERNEL_TRACE_CORES", "0")
        kwargs.update(trace=True,
                      trace_cores=[int(c) for c in cores.split(",")])

    res = run_bass_kernel_spmd(nc, in_maps, core_ids=list(range(NCORES)),
                               **kwargs)
    LAST_RESULT = res
    outs = [res.results[c]["out"].reshape(-1) for c in range(NCORES)]
    Pk = np.concatenate(outs[:TP]).reshape(X, X).astype(np.float32)
    Sk = np.concatenate(outs[TP:]).reshape(Y, Y).astype(np.float32)
    return Pk, Sk



# revision 23
# speedup vs baseline: 1.3686x; 1.3686x over previous
"""Trainium2 Bass kernel for SKalmanNet GSS (dense GEMV chain, batch=1).

Strategy (8 NeuronCores, 2 branches x 4-way tensor parallel):
  - Cores 0-3 compute Pk (l1/gru1/l2), cores 4-7 compute Sk (replica
    groups [[0,1,2,3],[4,5,6,7]]), one SPMD program.
  - Weight dtypes: l1 / gru Wih / gru Whh in fp8-e3m4 (per-tensor
    power-of-2 scale folded into the host-prepped activations), l2_W1 /
    l2_W2 in fp16.  Measured end-to-end rel err ~1.1e-2 (tol 2e-2).
  - Collective layout (2 collectives instead of 3):
      l1   row-sharded    -> AllGather(l1_out)        [collective 1]
      gru  row-sharded    -> h' shard stays local
      l2_W1 COLUMN-sharded by h-dims -> partial [4096] -> ReduceScatter
                                                         [collective 2]
      l2_W2 COLUMN-sharded by the RS output chunk -> partial [1024] out
      host sums the 4 partial outputs per branch (+ b2).
  - All weights are SBUF-resident (no ring reuse): DMA streams ~18.8MB
    per core with zero stalls.  wl1 + collective staging ride the ACT
    HWDGE ring so the l1->AllGather trigger path never queues behind
    the bulk weight stream (SP ring).
  - GEMVs run 3 PE column-group "chains" (tile_position col offsets
    0/32/64): 3 concurrent matmul streams (~2.4x PE throughput), one
    m-tile per chain, K-accumulated in PSUM, no cross-chain merge.
  - Elementwise stages avoid [1,N] single-lane layouts: psum quadrants
    are copied in one [65,512] op, gate math runs partition-major
    [128,4] after tiny K=1 matmul transposes.
"""

import os

import numpy as np

X = 32
Y = 32
H1 = 5120
HID = 2048
H2 = 4096
OUT = 1024
IN = 1120
INP = 1152          # 9*128, slot 1120 = bias

NCORES = 8
TP = 4

M_L1 = H1 // TP     # 1280
M_G = 3 * (HID // TP)   # 1536 (r|z|n x 512)
HSH = HID // TP     # 512
Y3C = H2 // TP      # 1024 (ReduceScatter output chunk)

K_L1 = INP // 128   # 9
K_IH = H1 // 128    # 40
K_HH = HID // 128   # 16
K_W1 = HSH // 128   # 4
K_W2 = Y3C // 128   # 8

PL1 = 1536          # padded AllGather width (48*32, xbar-transposable)

# partition-major consts [128, CPM_TOT] f32: each [512] vec as [128,4]
CPM_BRZ_R = 0       # bih+bhh, r gate
CPM_BRZ_Z = 4
CPM_BHHN = 8
CPM_BIHN = 12
CPM_HSH = 16
CPM_B1 = 20         # l2_b1 RS chunk as [128, 8]
CPM_TOT = 28

FP8 = os.environ.get("KERNEL_FP8", "1") == "1"
FMAX = 12.0         # target max for e3m4 scaling (format max 15.5)

_CACHE = {}


def _build_nc(s_ih):
    import concourse.bass as bass  # noqa: F401
    import concourse.mybir as mybir
    import concourse.tile as tile
    from concourse import bacc

    f32 = mybir.dt.float32
    f16 = mybir.dt.float16
    wq = mybir.dt.float8e3 if FP8 else mybir.dt.float16

    nc = bacc.Bacc("TRN2", target_bir_lowering=False, debug=False,
                   num_devices=NCORES)

    x0_d = nc.dram_tensor("x0", [128, K_L1], f16, kind="ExternalInput")
    hx_d = nc.dram_tensor("hx", [128, K_HH], f16, kind="ExternalInput")
    cpm_d = nc.dram_tensor("cpm", [128, CPM_TOT], f32, kind="ExternalInput")
    wl1_d = nc.dram_tensor("wl1", [3, 128, 3 * M_L1], wq, kind="ExternalInput")
    wih_d = nc.dram_tensor("wih", [4, 128, 10 * M_G], wq,
                           kind="ExternalInput")
    whh_d = nc.dram_tensor("whh", [1, 128, K_HH * M_G], wq,
                           kind="ExternalInput")
    w1_d = nc.dram_tensor("w1", [1, 128, K_W1 * H2], f16,
                          kind="ExternalInput")
    w2_d = nc.dram_tensor("w2", [1, 128, K_W2 * OUT], f16,
                          kind="ExternalInput")
    out_d = nc.dram_tensor("out", [2, 512], f32, kind="ExternalOutput")

    AF = mybir.ActivationFunctionType
    ALU = mybir.AluOpType
    groups = [[0, 1, 2, 3], [4, 5, 6, 7]]

    with tile.TileContext(nc) as tc, \
         tc.tile_pool(name="w", bufs=1) as wpool, \
         tc.tile_pool(name="act", bufs=1) as apool, \
         tc.tile_pool(name="ps", bufs=8, space="PSUM") as ppool, \
         tc.tile_pool(name="dram", bufs=1, space="DRAM") as dpool:

        # ---- input / consts DMAs (SWDGE queue, parallel to weights)
        cpm = apool.tile([128, CPM_TOT], f32, tag="cpm", name="cpm_sb")
        nc.gpsimd.dma_start(cpm, cpm_d.ap())
        x0 = apool.tile([128, K_L1], f16, tag="x0", name="x0")
        nc.gpsimd.dma_start(x0, x0_d.ap())
        hx = apool.tile([128, K_HH], f16, tag="hx", name="hx")
        nc.gpsimd.dma_start(hx, hx_d.ap())

        # ---- resident weight buffers
        wl1 = wpool.tile([128, K_L1 * M_L1], wq, tag="wl1", name="wl1_sb")
        wih = wpool.tile([128, K_IH * M_G], wq, tag="wih", name="wih_sb")
        whh = wpool.tile([128, K_HH * M_G], wq, tag="whh", name="whh_sb")
        w1 = wpool.tile([128, K_W1 * H2], f16, tag="w1", name="w1_sb")
        w2 = wpool.tile([128, K_W2 * OUT], f16, tag="w2", name="w2_sb")
        # wl1 heads the SP-ring queue so l1 can start before the bulk
        # stream saturates HBM (both HWDGE rings share the SDMA engines,
        # so a separate ring gives no priority).
        for p in range(3):
            nc.sync.dma_start(wl1[:, p * 3 * M_L1:(p + 1) * 3 * M_L1],
                              wl1_d.ap()[p])
        nc.sync.dma_start(whh[:, :], whh_d.ap()[0])
        for p in range(4):
            nc.sync.dma_start(wih[:, p * 10 * M_G:(p + 1) * 10 * M_G],
                              wih_d.ap()[p])
        nc.sync.dma_start(w1[:, :], w1_d.ap()[0])
        nc.sync.dma_start(w2[:, :], w2_d.ap()[0])

        # ---- ACT LUT warmup
        warm = apool.tile([1, 32], f32, tag="warm", name="warm")
        nc.vector.memset(warm, 0.0)
        nc.scalar.activation(warm, warm, AF.Sigmoid)
        nc.scalar.activation(warm, warm, AF.Tanh)
        nc.scalar.activation(warm, warm, AF.Relu)

        ones = apool.tile([128, 1], f16, tag="ones", name="ones")
        nc.vector.memset(ones, 1.0)

        # ---- PSUM: 8 banks, allocated up-front.  The [65,512]-style
        # batched psum->SBUF copies read quadrant-gap rows, so zero those
        # regions early (off the critical path).
        l1p = ppool.tile([128, 512], f32, tag="l1p", bufs=1, name="l1p")
        ghp = ppool.tile([128, 512], f32, tag="ghp", bufs=1, name="ghp")
        gip = ppool.tile([128, 512], f32, tag="gip", bufs=1, name="gip")
        w1ps = [ppool.tile([128, 512], f32, tag=f"w1p{b}", bufs=1,
                           name=f"w1p{b}") for b in range(3)]
        w2p = ppool.tile([128, 512], f32, tag="w2p", bufs=1, name="w2p")
        smalls = ppool.tile([128, 32], f32, tag="smalls", bufs=1,
                            name="smalls")
        ghtp = smalls[:, 0:12]
        gtp = smalls[:, 12:24]
        x3ps = smalls[:, 24:32]
        for t in (l1p, ghp, gip, w1ps[0], w1ps[1]):
            nc.vector.memset(t[0:65, :], 0.0)
        nc.vector.memset(w1ps[2][0:33, :], 0.0)
        nc.vector.memset(w2p[0:33, :], 0.0)

        def gemv(x_sb, w_sb, K, M, acc, xmap=None):
            """acc[j//3][32*(j%3), :mw] = W @ x for m-tile j (512 wide).
            3 concurrent column-group chains, K-accumulated in psum.
            k-major: one LDW per chain per k, all m-tiles streamed."""
            nm = (M + 511) // 512
            mts = [(i * 512, min(512, M - i * 512)) for i in range(nm)]
            for k in range(K):
                kk = xmap(k) if xmap else k
                for j in range(nm):
                    m0, mw = mts[j]
                    c = 32 * (j % 3)
                    nc.tensor.matmul(
                        acc[j // 3][c:c + 1, :mw],
                        x_sb[:, kk:kk + 1],
                        w_sb[:, k * M + m0: k * M + m0 + mw],
                        start=(k == 0), stop=(k == K - 1),
                        tile_position=(0, c),
                    )
            return mts

        def to_part(src, col0, cols, ps, pcol0):
            """src[0, col0+128*c : col0+128*(c+1)] (fp16 SBUF, partition 0)
            -> ps[:, pcol0+c] for c in range(cols), via K=1 matmuls."""
            for c in range(cols):
                s0 = col0 + c * 128
                nc.tensor.matmul(ps[:, pcol0 + c:pcol0 + c + 1],
                                 src[0:1, s0:s0 + 128],
                                 ones[0:1, 0:1],
                                 start=True, stop=True)

        # ---- l1: relu(W@x)*s_ih, row-sharded (bias folded in weights)
        gemv(x0, wl1, K_L1, M_L1, [l1p])
        # y1 staged as [65, 512] fp16; only rows 0/32/64 (the m-tiles) are
        # consumed, the quadrant-gap rows hold garbage and are never read.
        y1 = apool.tile([65, 512], f16, tag="y1", name="y1")
        nc.scalar.activation(y1, l1p[0:65, :], AF.Relu, scale=float(s_ih))
        nc.vector.memset(y1[64:65, 256:512], 0.0)

        # ---- AllGather l1_out (single partition-strided staging DMA)
        agin = dpool.tile([3, 512], f16, tag="agin", name="agin")
        agout = dpool.tile([TP, PL1], f16, tag="agout", name="agout")
        nc.scalar.dma_start(agin, y1[0:65:32, :])
        nc.gpsimd.collective_compute(
            "AllGather", ALU.bypass, replica_groups=groups,
            ins=[agin.opt()], outs=[agout.opt()])
        x1 = apool.tile([128, TP * PL1 // 128], f16, tag="x1", name="x1")
        nc.scalar.dma_start_transpose(
            x1, agout.rearrange("r (k p) -> (r k) p", p=128))

        # ---- gru: gh = Whh @ (hn*s_hh); off the critical path (pre-AG)
        gemv(hx, whh, K_HH, M_G, [ghp])
        ghs = apool.tile([1, M_G], f16, tag="ghs", name="ghs")
        for g in range(3):
            nc.scalar.copy(ghs[:, 512 * g:512 * (g + 1)],
                           ghp[32 * g:32 * g + 1, :])
        to_part(ghs, 0, 12, ghtp, 0)
        # ghb = gh + [brz_r | brz_z | bhhn]  (partition-major [128,12])
        ghb = apool.tile([128, 12], f32, tag="ghb", name="ghb")
        nc.vector.tensor_add(ghb, ghtp, cpm[:, 0:12])

        xmap1 = lambda k: (k // 10) * (PL1 // 128) + k % 10  # noqa: E731
        gemv(x1, wih, K_IH, M_G, [gip], xmap=xmap1)
        gis = apool.tile([1, M_G], f16, tag="gis", name="gis")
        for g in range(3):
            nc.scalar.copy(gis[:, 512 * g:512 * (g + 1)],
                           gip[32 * g:32 * g + 1, :])
        to_part(gis, 0, 12, gtp, 0)

        # ---- gru cell elementwise, partition-major [128, 4] per gate
        rz = apool.tile([128, 8], f32, tag="rz", name="rz")
        nc.vector.tensor_add(rz, gtp[:, 0:8], ghb[:, 0:8])
        nc.scalar.activation(rz, rz, AF.Sigmoid)
        tn = apool.tile([128, 4], f32, tag="tn", name="tn")
        nc.vector.tensor_mul(tn, rz[:, 0:4], ghb[:, 8:12])      # r*(ghn+bhhn)
        tn2 = apool.tile([128, 4], f32, tag="tn2", name="tn2")
        nc.vector.tensor_add(tn2, gtp[:, 8:12], cpm[:, CPM_BIHN:CPM_BIHN + 4])
        nc.vector.tensor_add(tn, tn, tn2)
        nc.scalar.activation(tn, tn, AF.Tanh)                   # n
        td = apool.tile([128, 4], f32, tag="td", name="td")
        nc.vector.tensor_sub(td, cpm[:, CPM_HSH:CPM_HSH + 4], tn)  # h-n
        nc.vector.tensor_mul(td, rz[:, 4:8], td)                # z*(h-n)
        x2 = apool.tile([128, 4], f16, tag="x2", name="x2")
        nc.vector.tensor_add(x2, tn, td)                        # h' [128,4]

        # ---- l2_W1 column-sharded: partial[4096] = W1[:, shard] @ h'
        gemv(x2, w1, K_W1, H2, w1ps)
        rsin = dpool.tile([8, 512], f16, tag="rsin", name="rsin")
        rsout = dpool.tile([1, Y3C], f16, tag="rsout", name="rsout")
        for b in range(3):
            nmt = 3 if b < 2 else 2
            rows = 32 * (nmt - 1) + 1
            yb = apool.tile([65, 512], f16, tag=f"yb{b}", name=f"yb{b}")
            nc.scalar.copy(yb[0:rows, :], w1ps[b][0:rows, :])
            nc.scalar.dma_start(rsin[3 * b:3 * b + nmt, :], yb[0:rows:32, :])
        nc.gpsimd.collective_compute(
            "ReduceScatter", ALU.add, replica_groups=groups,
            ins=[rsin.opt()], outs=[rsout.opt()])
        y3p = apool.tile([1, Y3C], f16, tag="y3p", name="y3p")
        nc.scalar.dma_start(y3p, rsout)

        # transpose to [128, 8], then bias+relu in partition-major form
        to_part(y3p, 0, 8, x3ps, 0)
        x3t = apool.tile([128, 8], f32, tag="x3t", name="x3t")
        nc.vector.tensor_add(x3t, x3ps, cpm[:, CPM_B1:CPM_B1 + 8])
        x3 = apool.tile([128, 8], f16, tag="x3", name="x3")
        nc.scalar.activation(x3, x3t, AF.Relu)

        # ---- l2_W2 column-sharded: partial [1024] out
        gemv(x3, w2, K_W2, OUT, [w2p])
        yo = apool.tile([33, 512], f32, tag="yo", name="yo")
        nc.scalar.copy(yo, w2p[0:33, :])
        nc.scalar.dma_start(out_d.ap(), yo[0:33:32, :])

    nc.finalize()
    return nc


def _pow2_scale(*arrs):
    m = max(float(np.abs(a).max()) for a in arrs)
    return float(2.0 ** np.ceil(np.log2(max(m, 1e-30) / FMAX)))


def _qpack(wt, K, M, npieces, npw):
    """[K*128, M] input-major transposed weight -> [npieces, 128, K*M/np]
    chunk-major packed (element [p, k*M+m] = wt[k*128+p, m])."""
    v = (wt.reshape(K, 128, M).transpose(1, 0, 2)
         .reshape(128, npieces, K * M // npieces).transpose(1, 0, 2))
    return np.ascontiguousarray(v).astype(npw)


def _pm(vec):
    """[n*128] -> [128, n] partition-major (element u -> [u%128, u//128])."""
    return np.ascontiguousarray(np.asarray(vec, np.float32)
                                .reshape(-1, 128).T)


def _prep_core(r, xvec, hn, l1W, l1b, Wih, Whh, bih, bhh, W1, b1, W2, b2,
               s_l1, s_ih, s_hh, npw):
    f32 = np.float32
    f16 = np.float16

    rs = slice(r * M_L1, (r + 1) * M_L1)
    wt = np.zeros((INP, M_L1), f32)
    wt[:IN] = l1W[rs].T
    wt[IN] = l1b[rs]
    wl1 = _qpack(wt / s_l1, K_L1, M_L1, 3, npw)

    gsl = [slice(g * HID + r * HSH, g * HID + (r + 1) * HSH) for g in range(3)]
    gidx = np.concatenate([np.arange(s.start, s.stop) for s in gsl])
    wih = _qpack(np.ascontiguousarray(Wih[gidx].T) / s_ih, K_IH, M_G, 4, npw)
    whh = _qpack(np.ascontiguousarray(Whh[gidx].T) / s_hh, K_HH, M_G, 1, npw)

    w1 = _qpack(np.ascontiguousarray(W1[:, r * HSH:(r + 1) * HSH].T),
                K_W1, H2, 1, f16)
    w2 = _qpack(np.ascontiguousarray(W2[:, r * Y3C:(r + 1) * Y3C].T),
                K_W2, OUT, 1, f16)

    bsum = bih + bhh
    cpm = np.concatenate([
        _pm(bsum[gsl[0]]), _pm(bsum[gsl[1]]),      # brz_r, brz_z
        _pm(bhh[gsl[2]]), _pm(bih[gsl[2]]),        # bhhn, bihn
        _pm(hn[r * HSH:(r + 1) * HSH]),            # h shard
        _pm(b1[r * Y3C:(r + 1) * Y3C]),            # RS chunk bias
    ], axis=1).astype(f32)
    assert cpm.shape == (128, CPM_TOT)

    x = np.zeros(INP, f32)
    x[:IN] = xvec
    x[IN] = 1.0
    x0 = np.ascontiguousarray((x * s_l1).reshape(K_L1, 128).T).astype(f16)
    hx = np.ascontiguousarray((hn * s_hh).reshape(K_HH, 128).T).astype(f16)

    return {
        "x0": x0, "hx": hx, "cpm": cpm,
        "wl1": wl1, "wih": wih, "whh": whh, "w1": w1, "w2": w2,
    }


LAST_RESULT = None


def kernel(state_inno, observation_inno, diff_state, diff_obs,
           linearization_error, Jacobian,
           l1_W, l1_b, gru1_Wih, gru1_Whh, gru1_bih, gru1_bhh,
           l2_W1, l2_b1, l2_W2, l2_b2,
           l3_W, l3_b, gru2_Wih, gru2_Whh, gru2_bih, gru2_bhh,
           l4_W1, l4_b1, l4_W2, l4_b2, hn1, hn2):
    global LAST_RESULT
    from concourse.bass_utils import run_bass_kernel_spmd
    import concourse.mybir as mybir

    npw = mybir.dt.np(mybir.dt.float8e3) if FP8 else np.float16

    a = lambda v: np.asarray(v, dtype=np.float32)
    input1 = np.concatenate([a(state_inno), a(diff_state),
                             a(linearization_error), a(Jacobian)]).reshape(-1)
    input2 = np.concatenate([a(observation_inno), a(diff_obs),
                             a(linearization_error), a(Jacobian)]).reshape(-1)

    branches = [
        (input1, a(hn1).reshape(-1), a(l1_W), a(l1_b).reshape(-1),
         a(gru1_Wih), a(gru1_Whh), a(gru1_bih).reshape(-1),
         a(gru1_bhh).reshape(-1), a(l2_W1), a(l2_b1).reshape(-1),
         a(l2_W2), a(l2_b2).reshape(-1)),
        (input2, a(hn2).reshape(-1), a(l3_W), a(l3_b).reshape(-1),
         a(gru2_Wih), a(gru2_Whh), a(gru2_bih).reshape(-1),
         a(gru2_bhh).reshape(-1), a(l4_W1), a(l4_b1).reshape(-1),
         a(l4_W2), a(l4_b2).reshape(-1)),
    ]

    if FP8:
        s_l1 = _pow2_scale(
            np.concatenate([branches[0][2].ravel(), branches[0][3]]),
            np.concatenate([branches[1][2].ravel(), branches[1][3]]))
        s_ih = _pow2_scale(branches[0][4], branches[1][4])
        s_hh = _pow2_scale(branches[0][5], branches[1][5])
    else:
        s_l1 = s_ih = s_hh = 1.0

    if "nc" not in _CACHE:
        _CACHE["nc"] = (_build_nc(s_ih), s_l1, s_ih, s_hh)
    nc, s_l1, s_ih, s_hh = _CACHE["nc"]

    in_maps = [_prep_core(c % TP, *branches[c // TP],
                          s_l1, s_ih, s_hh, npw) for c in range(NCORES)]

    kwargs = {}
    if os.environ.get("KERNEL_TRACE"):
        cores = os.environ.get("KERNEL_TRACE_CORES", "0")
        kwargs.update(trace=True,
                      trace_cores=[int(c) for c in cores.split(",")])

    res = run_bass_kernel_spmd(nc, in_maps, core_ids=list(range(NCORES)),
                               **kwargs)
    LAST_RESULT = res
    outs = [res.results[c]["out"].reshape(-1) for c in range(NCORES)]
    b2P = branches[0][11]
    b2S = branches[1][11]
    Pk = (sum(outs[:TP]) + b2P).reshape(X, X).astype(np.float32)
    Sk = (sum(outs[TP:]) + b2S).reshape(Y, Y).astype(np.float32)
    return Pk, Sk
